# revision 75
# baseline (speedup 1.0000x reference)
"""Trainium2 Bass kernel for MultiHeadAttentionRoPE.

Problem (hardcoded): B=2, S=2048, D=1024, H=16 heads, Dh=64, fp32 I/O.
    qkv = x @ w_qkv ; q,k -> RoPE ; causal attention ; out = ctx @ w_proj

Sharding: tensor-parallel over heads across 8 cores (2 heads/core).
Each core reads the full x (bf16, transposed+tiled on host), its slice of
w_qkv/w_proj, computes attention for its 2 heads and a *partial*
projection output (bf16); the host sums the 8 partials in fp32.

Per-core design notes (v2):
  - Everything on-device is bf16 (fp32 PSUM accumulation), which halves
    DMA traffic and SBUF footprint and enables DVE 2x modes.
  - x is host-pre-tiled to [B, NTCH, KC, 128, TCH] so each (b,t) SBUF
    tile loads with ONE dma_start (HWDGE issue slots are the scarce
    resource: ~650ns serialized per DMA).
  - RoPE rotate-half is a single DVE stream_shuffle: the feature->
    partition mapping is permuted host-side (weight columns + cos/sin
    rows) so that rotate pairs live in the same 32-partition quadrant.
    No SBUF->SBUF DMAs.
  - v is produced directly in natural (token, feature) layout by using
    the x-tile as the matmul's stationary operand (no PE transposes).
  - Scores are computed transposed (keys on partitions); softmax
    denominator comes free from ones-columns in vt via the PV matmul.
    h1's PV accumulates at partitions 63:128 so every normalization and
    writeback op is partition-lockstep (engines can't shift partitions).
  - Causal band masking multiplies by a 0/1 triangle on the (otherwise
    idle) Pool engine; fully-masked column prefixes are skipped.
  - attention(b=0) is interleaved with stage1(b=1) chunk-by-chunk in
    program order: engine queues are in-order, so interleaving is what
    actually lets the PE run ahead while ACT does exps.
"""

import functools
import os
import sys

import numpy as np

sys.path.insert(0, "/opt/trn_rl_repo")

# ---- problem constants (must match reference.py) ----
B = 2
S = 2048
D = 1024
H = 16
Dh = 64
N_CORES = 8
HPC = H // N_CORES          # heads per core = 2
KC = D // 128               # contraction chunks = 8
TCH = 512                   # token chunk
NTCH = S // TCH             # 4 chunks per batch
NSUB = S // 128             # 16 key subchunks per batch
ROPE_BASE = 10000.0
SCALE = 1.0 / 8.0           # 1/sqrt(Dh)

# feature permutation within one head's 64 partition rows: rotate pairs
# (f, f+32) land in the same 32-partition quadrant so stream_shuffle can
# swap them.  P64[row] = original feature index held by that row.
P64 = list(range(0, 16)) + list(range(32, 48)) + list(range(16, 32)) + list(range(48, 64))
# within-quadrant swap mask: dest i <- source (i+16)%32
SHUF_MASK = [(i + 16) % 32 for i in range(32)]


def _build_program(loop_n=1, phases="all", opts=""):
    import concourse.bass as bass
    opts = set(opts.split(",")) if opts else set()
    import concourse.mybir as mybir
    import concourse.tile as tile
    from concourse import bacc
    from contextlib import ExitStack

    FP = mybir.dt.float32
    BF = mybir.dt.bfloat16
    FPR = mybir.dt.float32r
    EXP = mybir.ActivationFunctionType.Exp
    R = lambda ap: ap.bitcast(FPR)

    nc = bacc.Bacc("TRN2", target_bir_lowering=False, debug=False)

    xt_d = nc.dram_tensor("xt", [B, NTCH, KC, 128, TCH], BF, kind="ExternalInput").ap()
    wqk_d = nc.dram_tensor("wqk", [KC, 128, 3 * 128], BF, kind="ExternalInput").ap()
    wproj_d = nc.dram_tensor("wproj", [128, D], BF, kind="ExternalInput").ap()
    cos_d = nc.dram_tensor("cost", [128, S], BF, kind="ExternalInput").ap()
    sin_d = nc.dram_tensor("sint", [128, S], BF, kind="ExternalInput").ap()
    tri_d = nc.dram_tensor("tri", [128, 2, 128], BF, kind="ExternalInput").ap()
    ident_d = nc.dram_tensor("ident", [128, 128], BF, kind="ExternalInput").ap()
    ones2_d = nc.dram_tensor("ones2", [128, NSUB, 2], BF, kind="ExternalInput").ap()
    onesb_d = nc.dram_tensor("onesb", [128, 64], FP, kind="ExternalInput").ap()
    out_d = nc.dram_tensor("out", [B, S, D], BF, kind="ExternalOutput").ap()

    with tile.TileContext(nc) as tc, ExitStack() as ctx:
        consts = ctx.enter_context(tc.tile_pool(name="consts", bufs=1))
        store = ctx.enter_context(tc.tile_pool(name="store", bufs=1))
        xt_pool = ctx.enter_context(tc.tile_pool(name="xt_pool", bufs=3))
        rope_pool = ctx.enter_context(tc.tile_pool(name="rope_pool", bufs=3))
        p_pool = ctx.enter_context(tc.tile_pool(name="p_pool", bufs=8))
        nrm_pool = ctx.enter_context(tc.tile_pool(name="nrm_pool", bufs=3))
        ob_pool = ctx.enter_context(tc.tile_pool(name="ob_pool", bufs=4))
        # PSUM budget (8 banks): ps_a 2 (stage1 q/k/v + transposes + proj),
        # ps_s 4 (two-bank merged-head score tiles + pbc), ps_pv 2.
        ps_a = ctx.enter_context(tc.tile_pool(name="ps_a", bufs=2, space="PSUM"))
        ps_s = ctx.enter_context(tc.tile_pool(name="ps_s", bufs=2, space="PSUM"))
        ps_pv = ctx.enter_context(tc.tile_pool(name="ps_pv", bufs=2, space="PSUM"))

        # ---- constants ----
        wqk_sb = consts.tile([128, KC, 384], BF, name="wqk_sb")
        wproj_sb = consts.tile([128, D], BF, name="wproj_sb")
        cos_sb = consts.tile([128, S], BF, name="cos_sb")
        sin_sb = consts.tile([128, S], BF, name="sin_sb")
        tri_sb = consts.tile([128, 2, 128], BF, name="tri_sb")
        ident_sb = consts.tile([128, 128], BF, name="ident_sb")
        onesb_sb = consts.tile([128, 64], FP, name="onesb_sb")

        def late_consts():
            nc.sync.dma_start(cos_sb, cos_d)
            nc.sync.dma_start(sin_sb, sin_d)
            nc.sync.dma_start(tri_sb, tri_d)
            nc.sync.dma_start(ident_sb, ident_d)
            nc.sync.dma_start(wproj_sb, wproj_d)
            nc.sync.dma_start(R(onesb_sb), R(onesb_d))
            for bb in range(B):
                nc.sync.dma_start(vt[bb][:, :, 64:65], ones2_d[:, :, 0:1])
                nc.sync.dma_start(vt[bb][:, :, 129:130], ones2_d[:, :, 1:2])

        # ---- persistent per-batch storage ----
        qT = {}   # (b, t) -> (128, 512) bf16; rows: [h0 perm-feats | h1 perm-feats]
        kT = {}
        vt = {}   # b -> (128, NSUB, 130) bf16; cols [v_h0(0:64) | 1 | v_h1(65:129) | 1]
        ctxT = {}
        for b in range(B):
            ctxT[b] = store.tile([128, S], BF, name=f"ctxT_{b}", tag=f"ctxT_{b}")
            for t in range(NTCH):
                qT[b, t] = store.tile([128, TCH], BF, name=f"qT_{b}_{t}", tag=f"qT_{b}_{t}")
                kT[b, t] = store.tile([128, TCH], BF, name=f"kT_{b}_{t}", tag=f"kT_{b}_{t}")
            vt[b] = store.tile([128, NSUB, 130], BF, name=f"vt_{b}", tag=f"vt_{b}")

        def stage1_t(b, t, first=False):
            """QKV projection + RoPE + v natural layout for chunk t of batch b."""
            tsl = slice(t * TCH, (t + 1) * TCH)
            xtile = xt_pool.tile([128, KC, TCH], BF, name="xtile", tag="xt")
            if first:
                # split the very first x-tile so the kc=0/1 matmuls can
                # start before the rest of the tile lands
                nc.sync.dma_start(
                    xtile[:, 0:2, :], xt_d[b, t, 0:2].rearrange("k p f -> p k f")
                )
                nc.sync.dma_start(
                    xtile[:, 2:KC, :], xt_d[b, t, 2:KC].rearrange("k p f -> p k f")
                )
            else:
                nc.sync.dma_start(xtile, xt_d[b, t].rearrange("k p f -> p k f"))
            if first:
                # wqk split so the kc=0/1 matmuls can start before the full load
                nc.sync.dma_start(
                    wqk_sb[:, 0:2, :], wqk_d[0:2].rearrange("k p f -> p k f")
                )
                nc.sync.dma_start(
                    wqk_sb[:, 2:KC, :], wqk_d[2:KC].rearrange("k p f -> p k f")
                )
                late_consts()

            # --- q, k (transposed layout, RoPE via quadrant shuffle) ---
            for fc in range(2):
                psq = ps_a.tile([128, TCH], FP, name="psq", tag="ps_a")
                for kc in range(KC):
                    nc.tensor.matmul(
                        psq,
                        lhsT=wqk_sb[:, kc, fc * 128:(fc + 1) * 128],
                        rhs=xtile[:, kc, :],
                        start=(kc == 0),
                        stop=(kc == KC - 1),
                    )
                dest = qT[b, t] if fc == 0 else kT[b, t]
                qc = rope_pool.tile([128, TCH], BF, name="qc", tag="qc")
                nc.scalar.copy(qc, psq)
                qs = rope_pool.tile([128, TCH], BF, name="qs", tag="qs")
                nc.vector.stream_shuffle(qs, qc, SHUF_MASK)
                t1 = rope_pool.tile([128, TCH], BF, name="t1", tag="t1")
                nc.vector.tensor_mul(out=t1, in0=qs, in1=sin_sb[:, tsl])
                nc.vector.tensor_mul(out=dest, in0=qc, in1=cos_sb[:, tsl])
                nc.vector.tensor_add(out=dest, in0=dest, in1=t1)

            # --- v: transposed matmul (wv stationary, one weight load per
            # kc) then PE transposes to natural (token, feature) layout ---
            psv = ps_a.tile([128, TCH], FP, name="psv", tag="ps_a")
            for kc in range(KC):
                nc.tensor.matmul(
                    psv,
                    lhsT=wqk_sb[:, kc, 256:384],
                    rhs=xtile[:, kc, :],
                    start=(kc == 0),
                    stop=(kc == KC - 1),
                )
            vts = rope_pool.tile([128, TCH], BF, name="vts", tag="vts")
            nc.scalar.copy(vts, psv)
            pv4 = ps_a.tile([128, 4, 128], BF, name="pv4", tag="ps_a")
            for sc4 in range(4):
                nc.tensor.transpose(
                    pv4[:, sc4, :], vts[:, sc4 * 128:(sc4 + 1) * 128], ident_sb
                )
            nc.vector.tensor_copy(
                out=vt[b][:, 4 * t:4 * t + 4, 0:64], in_=pv4[:, :, 0:64]
            )
            nc.vector.tensor_copy(
                out=vt[b][:, 4 * t:4 * t + 4, 65:129], in_=pv4[:, :, 64:128]
            )

        def attention_qb(b, qb, carry=(), tail=False):
            """Causal attention for both heads, query chunk qb.

            `carry` holds the previous chunk's projection emitters; they are
            interleaved one-per-kc into this chunk's loop so the (in-order)
            PE queue has exp-independent matmuls to chew on while ACT works.
            Returns this chunk's own projection emitters."""
            qsl = slice(qb * TCH, (qb + 1) * TCH)
            nkc = 4 * qb + 4
            # per head: ctx rows 0:64 + den row 64 (matmul out base must be 0/32/64)
            pv = [
                ps_pv.tile([128, TCH], FP, name=f"pv{h}", tag="ps_pv")
                for h in range(HPC)
            ]
            pend = []  # software-pipelined PV args (lag ~2 kc behind scores)
            carry = list(carry)

            def flush_pv(keep=0):
                while len(pend) > keep:
                    pv_out, lhsT, rhs, st, sp = pend.pop(0)
                    nc.tensor.matmul(pv_out, lhsT=lhsT, rhs=rhs, start=st, stop=sp)

            for kc in range(nkc):
                if carry and kc > 0:
                    carry.pop(0)()
                off = max(0, (kc - 4 * qb) * 128)
                nv = TCH - off
                # both heads' scores in one 2-bank tile -> single merged exp
                ps = ps_s.tile([128, 2, TCH], FP, name="ps", tag="ps_s")
                for h in range(HPC):
                    hb = h * 64
                    nc.tensor.matmul(
                        ps[:, h, :nv],
                        lhsT=kT[b, kc // 4][hb:hb + 64, (kc % 4) * 128:(kc % 4 + 1) * 128],
                        rhs=qT[b, qb][hb:hb + 64, off:TCH],
                        start=True,
                        stop=True,
                    )
                p = p_pool.tile([128, 2, TCH], BF, name="p", tag="p")
                nc.scalar.activation(p[:, :, off:TCH], ps[:, :, :nv], EXP, scale=SCALE)
                if kc >= 4 * qb:  # diagonal band: triangular mask, both heads
                    nc.vector.tensor_mul(
                        out=p[:, :, off:off + 128],
                        in0=p[:, :, off:off + 128],
                        in1=tri_sb,
                    )
                for h in range(HPC):
                    pv_out = pv[h][0:65, off:TCH]
                    lhsT = vt[b][:, kc, 65 * h:65 * h + 65]
                    pend.append((pv_out, lhsT, p[:, h, off:TCH],
                                 kc == 0, kc == nkc - 1))
                flush_pv(keep=6)
            flush_pv()
            for emit in carry:
                emit()

            # normalize: 1/den row, broadcast across partitions via K=1 matmul
            for h in range(HPC):
                rcpS = nrm_pool.tile([128, TCH], FP, name="rcpS", tag="rcpS")
                with nc.allow_low_precision(reason="fp32r rounding for bcast matmul"):
                    nc.vector.reciprocal(R(rcpS[64:65, :]), pv[h][64:65, :])
                pbc = ps_s.tile([128, TCH], FP, name="pbc", tag="ps_s")
                nc.tensor.matmul(
                    pbc[0:64, :], lhsT=R(onesb_sb[64:65, :]), rhs=R(rcpS[64:65, :]),
                    start=True, stop=True,
                )
                rcb = nrm_pool.tile([64, TCH], FP, name="rcb", tag="rcb")
                if tail or h == 1:
                    nc.scalar.copy(rcb, pbc[0:64, :])
                else:
                    nc.vector.tensor_copy(out=rcb, in_=pbc[0:64, :])
                if h == 0:
                    nc.vector.tensor_mul(
                        out=ctxT[b][0:64, qsl], in0=pv[h][0:64, :], in1=rcb
                    )
                else:
                    ctmp = nrm_pool.tile([64, TCH], BF, name="ctmp", tag="ctmp")
                    nc.vector.tensor_mul(out=ctmp, in0=pv[h][0:64, :], in1=rcb)
                    nc.gpsimd.dma_start(ctxT[b][64:128, qsl], ctmp)

            # projection emitters for this qb's token range.  PSUM can't be
            # a DMA source, so the drain copies go through SBUF; spread them
            # across Pool/ACT/DVE to keep ACT free for exps.
            def make_proj(tb):
                def emit():
                    po0 = ps_a.tile([128, TCH], FP, name="po0", tag="ps_a")
                    nc.tensor.matmul(
                        po0, lhsT=ctxT[b][:, tb * 128:(tb + 1) * 128],
                        rhs=wproj_sb[:, 0:512], start=True, stop=True,
                    )
                    po1 = ps_a.tile([128, TCH], FP, name="po1", tag="ps_a")
                    nc.tensor.matmul(
                        po1, lhsT=ctxT[b][:, tb * 128:(tb + 1) * 128],
                        rhs=wproj_sb[:, 512:1024], start=True, stop=True,
                    )
                    ob = ob_pool.tile([128, 1024], BF, name="ob", tag="ob")
                    nc.vector.tensor_copy(out=ob[:, 0:512], in_=po0)
                    if tb % 4 == 0:
                        nc.scalar.copy(ob[:, 512:1024], po1)
                    else:
                        nc.vector.tensor_copy(out=ob[:, 512:1024], in_=po1)
                    nc.sync.dma_start(out_d[b, tb * 128:(tb + 1) * 128, :], ob)
                return emit

            return [make_proj(tb) for tb in range(4 * qb, 4 * qb + 4)]

        def whole():
            for t in range(NTCH):
                stage1_t(0, t, first=(t == 0))
            if phases == "s1":
                for t in range(NTCH):
                    stage1_t(1, t)
                return
            # interleave attention(0) with stage1(1) chunk-by-chunk; each
            # chunk's projection is deferred into the next chunk's kc loop.
            carry = ()
            for q in range(NTCH):
                carry = attention_qb(0, q, carry)
                stage1_t(1, q)
            # end with q0 (shortest kc loop) so the exposed tail is minimal
            for q in (1, 2, 3, 0):
                carry = attention_qb(1, q, carry, tail=(q == 0))
            for emit in carry:
                emit()

        if loop_n == 1:
            whole()
        else:
            with tc.For_i(0, loop_n, 1):
                whole()

    nc.compile()
    return nc


@functools.lru_cache(maxsize=4)
def _get_program(loop_n=1, phases="all", opts=""):
    return _build_program(loop_n, phases, opts)


def _host_inputs(x, w_qkv, w_proj):
    """Build the 8 per-core input maps from the full problem inputs."""
    import ml_dtypes
    BF = ml_dtypes.bfloat16

    x = np.asarray(x, dtype=np.float32)
    w_qkv = np.asarray(w_qkv, dtype=np.float32)
    w_proj = np.asarray(w_proj, dtype=np.float32)

    # x transposed + tiled: [B, NTCH, KC, 128, TCH]
    xt = np.ascontiguousarray(
        x.transpose(0, 2, 1)                      # (B, D, S)
        .reshape(B, KC, 128, NTCH, TCH)
        .transpose(0, 3, 1, 2, 4)                 # (B, NTCH, KC, 128, TCH)
    ).astype(BF)

    # RoPE tables in permuted row order, sin sign-folded.
    inv_freq = 1.0 / (ROPE_BASE ** (np.arange(0, Dh, 2, dtype=np.float32) / Dh))
    tpos = np.arange(S, dtype=np.float32)
    freqs = np.outer(tpos, inv_freq)                       # (S, 32)
    cos_f = np.cos(np.concatenate([freqs, freqs], -1)).T   # (64, S), row=feature
    sin_f = np.sin(np.concatenate([freqs, freqs], -1)).T
    sin_f = sin_f.copy()
    sin_f[:32] *= -1.0                                     # fold rotate_half sign
    p64 = np.array(P64)
    cos_perm = cos_f[p64]                                  # row r holds feature P64[r]
    sin_perm = sin_f[p64]
    cos_full = np.ascontiguousarray(np.tile(cos_perm, (2, 1))).astype(BF)
    sin_full = np.ascontiguousarray(np.tile(sin_perm, (2, 1))).astype(BF)

    r = np.arange(128)
    tri1 = (r[None, :] >= r[:, None]).astype(np.float32).astype(BF)
    tri = np.ascontiguousarray(np.stack([tri1, tri1], axis=1))  # (128, 2, 128)

    wq = w_qkv[:, 0:D]
    wk = w_qkv[:, D:2 * D]
    wv = w_qkv[:, 2 * D:3 * D]

    in_maps = []
    for c in range(N_CORES):
        h0, h1 = 2 * c, 2 * c + 1
        cols_n = np.r_[h0 * 64:(h0 + 1) * 64, h1 * 64:(h1 + 1) * 64]  # natural
        cols_p = np.r_[h0 * 64 + p64, h1 * 64 + p64]                  # permuted
        wqk_c = np.concatenate(
            [wq[:, cols_p], wk[:, cols_p], wv[:, cols_n]], axis=1
        )  # (D, 384)
        in_maps.append({
            "xt": xt,
            "wqk": np.ascontiguousarray(wqk_c).astype(BF).reshape(KC, 128, 384),
            "wproj": np.ascontiguousarray(w_proj[c * 128:(c + 1) * 128, :]).astype(BF),
            "cost": cos_full,
            "sint": sin_full,
            "tri": tri,
            "ident": np.eye(128, dtype=np.float32).astype(BF),
            "ones2": np.ones((128, NSUB, 2), dtype=BF),
            "onesb": np.ones((128, 64), dtype=np.float32),
        })
    return in_maps


_last_results = None


def kernel(x, w_qkv, w_proj):
    global _last_results
    from concourse.bass_utils import run_bass_kernel_spmd

    nc = _get_program()
    in_maps = _host_inputs(x, w_qkv, w_proj)
    trace = bool(int(os.environ.get("KERNEL_TRACE", "0")))
    kwargs = {}
    if trace:
        kwargs["trace"] = True
        kwargs["trace_cores"] = list(range(N_CORES))
    res = run_bass_kernel_spmd(nc, in_maps, core_ids=list(range(N_CORES)), **kwargs)
    _last_results = res
    acc = np.zeros((B, S, D), dtype=np.float64)
    for r in res.results:
        acc += np.asarray(r["out"], dtype=np.float64)
    return acc.astype(np.float32)
